# revision 17
# baseline (speedup 1.0000x reference)
"""GroupQueryAttention kernel for 8 Trainium2 NeuronCores.

Problem: B=2, S=2048, E=2048, H=16 heads, G=4 kv-groups, head_dim=128.

Sharding: batch x kv-group. Core d = (batch b = d//4, group g = d%4) owns
the 4 heads of group g for batch b: the 512-column slice of Wq, the
128-column slice of Wk/Wv, and the matching 512-row slice of Wo. This is
the even split of the model's 77.3e9 MACs: 9.67e9 MACs/core (~247us of
PE time at 1 col/cycle). Each core reads x[b]^T only (contraction dim on
partitions) and writes a partial y^T[b] that the host sums over the 4
group-cores of that batch (+bo, transpose).

dtypes: x/weights/q/k/attn/V/o in bf16 (same PE rate as f32r, half the
DMA and SBUF), psum accumulation and y partials f32.

The PE instruction stream is hand-interleaved. Attention iteration i
emits its score matmuls pair-by-pair with the AV matmuls of iteration
i-1, and a fine-grained "pump" drips filler matmuls (deferred Q
projections, Wo output-projection tiles) into every slot so the in-order
PE queue never stalls while the Act engine drains exp()s (Act needs
~1.04us per [128,1024] exp vs 427ns PE fill per score pair). The softmax
denominator is built incrementally on DVE from bf16 pair-sums so the
last iteration's normalization chain is short.
"""

import math

import numpy as np

B = 2
S = 2048
E = 2048
HD = 128
HLOC = 4  # heads per core (one kv group)
NGROUPS = 4
NCORES = 8
ECH = 16  # e-chunks of 128 for contraction
SC = 512  # s-chunk width for projections / Wo moving dim
NSC = S // SC  # 4
QC = 512  # q-chunk width in attention
NQC = S // QC  # 4
KJT = S // 128  # 16 kj tiles
PAIRS = KJT // 2  # 8 score-psum pairs per iteration
INV_SQRT_HD = 1.0 / math.sqrt(HD)

_CACHE = {}


def _build():
    import concourse.bacc as bacc
    import concourse.mybir as mybir
    import concourse.tile as tile

    f32 = mybir.dt.float32
    bf16 = mybir.dt.bfloat16
    AF = mybir.ActivationFunctionType
    ALU = mybir.AluOpType

    nc = bacc.Bacc("TRN2", target_bir_lowering=False, debug=False)

    xT = nc.dram_tensor("xT", [E, S], bf16, kind="ExternalInput").ap()
    wq = nc.dram_tensor("wq", [E, HLOC * HD], bf16, kind="ExternalInput").ap()
    bq = nc.dram_tensor("bq", [HLOC * HD], f32, kind="ExternalInput").ap()
    wk = nc.dram_tensor("wk", [E, HD], bf16, kind="ExternalInput").ap()
    bk = nc.dram_tensor("bk", [HD], f32, kind="ExternalInput").ap()
    wv = nc.dram_tensor("wv", [E, HD], bf16, kind="ExternalInput").ap()
    bvr = nc.dram_tensor("bvr", [1, HD], bf16, kind="ExternalInput").ap()
    wo = nc.dram_tensor("wo", [HLOC * HD, E], bf16, kind="ExternalInput").ap()
    yT = nc.dram_tensor("yT", [E, S], bf16, kind="ExternalOutput").ap()

    import bass_rust  # noqa: F401
    from concourse import bass_isa

    xTr = xT.rearrange("(t p) s -> p t s", p=128)
    yTr = yT.rearrange("(t p) s -> p t s", p=128)

    with tile.TileContext(nc) as tc:
        with (
            tc.tile_pool(name="pers", bufs=1) as pers,
            tc.tile_pool(name="xt", bufs=3) as xpool,
            tc.tile_pool(name="proj", bufs=1) as projp,
            tc.tile_pool(name="attn", bufs=2) as apool,
            tc.tile_pool(name="soft", bufs=1) as spool,
            tc.tile_pool(name="yst", bufs=3) as ypool,
            tc.tile_pool(name="ps_pp", bufs=2, space="PSUM") as pp,
            tc.tile_pool(name="ps_sc", bufs=2, space="PSUM") as psc,
            tc.tile_pool(name="ps_o", bufs=2, space="PSUM") as po,
        ):
            # --- persistent tiles ---
            wk_sb = pers.tile([128, ECH, HD], bf16)
            wv_sb = pers.tile([128, ECH, HD], bf16)
            bk_sb = pers.tile([128, 1], f32)
            bvr_sb = pers.tile([1, HD], bf16)
            wq_sb = pers.tile([128, ECH, HLOC * HD], bf16)
            bq_sb = pers.tile([128, HLOC], f32)
            wo_sb = pers.tile([128, HLOC, E], bf16)
            ones_sb = pers.tile([1, 128], bf16)
            nc.vector.memset(ones_sb, 1.0)

            # --- per-core activations ---
            qt = projp.tile([128, HLOC, S], bf16, tag="qt")
            kt = projp.tile([128, S], bf16, tag="kt")
            v_sb = projp.tile([128, KJT, HD], bf16, tag="v")
            ot = projp.tile([128, HLOC, S], bf16, tag="ot")

            copy_flip = [0]

            def psum_copy(dst, src):
                if copy_flip[0] % 2 == 0:
                    nc.scalar.copy(dst, src)
                else:
                    nc.vector.tensor_copy(dst, src)
                copy_flip[0] += 1

            xts = {}

            def load_x(sc, halves=1):
                t = xpool.tile([128, ECH, SC], bf16, tag="xt", name="xt")
                hh = ECH // halves
                for u in range(halves):
                    nc.sync.dma_start(
                        out=t[:, u * hh : (u + 1) * hh, :],
                        in_=xTr[:, u * hh : (u + 1) * hh, sc * SC : (sc + 1) * SC],
                    )
                xts[sc] = t

            def q_proj(h, qcn):
                ps = pp.tile([128, SC], f32, tag="pp", name="ps_q")
                xt_t = xts[qcn]
                for t in range(ECH):
                    nc.tensor.matmul(
                        ps,
                        lhsT=wq_sb[:, t, h * HD : (h + 1) * HD],
                        rhs=xt_t[:, t, :],
                        start=(t == 0),
                        stop=(t == ECH - 1),
                    )
                nc.scalar.activation(
                    qt[:, h, qcn * SC : (qcn + 1) * SC], ps, AF.Identity,
                    bias=bq_sb[:, h : h + 1],
                )

            def kv_unit(sc):
                xt_t = xts[sc]
                s0 = sc * SC
                ps = pp.tile([128, SC], f32, tag="pp", name="ps_k")
                for t in range(ECH):
                    nc.tensor.matmul(
                        ps,
                        lhsT=wk_sb[:, t, :],
                        rhs=xt_t[:, t, :],
                        start=(t == 0),
                        stop=(t == ECH - 1),
                    )
                nc.scalar.activation(
                    kt[:, s0 : s0 + SC], ps, AF.Identity, bias=bk_sb[:, 0:1]
                )
                # V directly in [s, hd] layout: x-tile is lhsT, wv is rhs;
                # bv folded in via a ones-row matmul (bias varies along the
                # free axis here, so the Act bias port can't add it).
                for j in range(SC // 128):
                    st = sc * (SC // 128) + j
                    psv = pp.tile([128, SC], f32, tag="pp", name="ps_v")
                    for t in range(ECH):
                        nc.tensor.matmul(
                            psv[:, 0:HD],
                            lhsT=xt_t[:, t, j * 128 : (j + 1) * 128],
                            rhs=wv_sb[:, t, :],
                            start=(t == 0),
                            stop=False,
                        )
                    nc.tensor.matmul(
                        psv[:, 0:HD], lhsT=ones_sb, rhs=bvr_sb,
                        start=False, stop=True,
                    )
                    nc.scalar.copy(v_sb[:, st, :], psv[:, 0:HD])

            # --- phase 1: K/V for all of S, Q for chunks 0-1 ---
            # DMA issue order is tuned so each consumer's data lands just
            # ahead of its matmuls (single SP HWDGE queue = bus order).
            xt0 = xpool.tile([128, ECH, SC], bf16, tag="xt", name="xt0")
            xts[0] = xt0
            wkr = wk.rearrange("(t p) m -> p t m", p=128)
            nc.sync.dma_start(out=xt0[:, 0:2, :], in_=xTr[:, 0:2, 0:SC])
            nc.sync.dma_start(out=wk_sb[:, 0:2, :], in_=wkr[:, 0:2, :])
            nc.sync.dma_start(out=xt0[:, 2:4, :], in_=xTr[:, 2:4, 0:SC])
            nc.sync.dma_start(out=wk_sb[:, 2:4, :], in_=wkr[:, 2:4, :])
            nc.sync.dma_start(out=xt0[:, 4:8, :], in_=xTr[:, 4:8, 0:SC])
            nc.sync.dma_start(out=wk_sb[:, 4:16, :], in_=wkr[:, 4:16, :])
            nc.sync.dma_start(out=xt0[:, 8:16, :], in_=xTr[:, 8:16, 0:SC])
            nc.sync.dma_start(out=wv_sb, in_=wv.rearrange("(t p) m -> p t m", p=128))
            nc.sync.dma_start(out=bk_sb, in_=bk.rearrange("(d o) -> d o", o=1))
            nc.sync.dma_start(out=bvr_sb, in_=bvr)
            load_x(1, halves=2)
            wqr = wq.rearrange("(t p) m -> p t m", p=128)
            nc.sync.dma_start(out=wq_sb[:, :, 0 : 2 * HD], in_=wqr[:, :, 0 : 2 * HD])
            nc.sync.dma_start(
                out=wq_sb[:, :, 2 * HD : 4 * HD], in_=wqr[:, :, 2 * HD : 4 * HD]
            )
            nc.sync.dma_start(out=bq_sb, in_=bq.rearrange("(h d) -> d h", d=128))

            kv_unit(0)
            load_x(2, halves=2)
            nc.sync.dma_start(out=wo_sb, in_=wo.rearrange("(h p) e -> p h e", p=128))
            kv_unit(1)
            for h in range(HLOC):
                q_proj(h, 0)
            kv_unit(2)
            load_x(3, halves=2)
            for h in range(HLOC):
                q_proj(h, 1)
            kv_unit(3)

            # --- phase 2: attention pipeline with pumped fillers ---
            iters = [(h, qc) for qc in range(NQC) for h in range(HLOC)]
            attn_tiles = {}
            pso_tiles = {}
            rec_tiles = {}

            def gen_q(h, qcn):
                def g():
                    ps = pp.tile([128, SC], f32, tag="pp", name="ps_qf")
                    xt_t = xts[qcn]
                    for t in range(ECH):
                        nc.tensor.matmul(
                            ps,
                            lhsT=wq_sb[:, t, h * HD : (h + 1) * HD],
                            rhs=xt_t[:, t, :],
                            start=(t == 0),
                            stop=(t == ECH - 1),
                        )
                        yield
                    nc.scalar.activation(
                        qt[:, h, qcn * SC : (qcn + 1) * SC], ps, AF.Identity,
                        bias=bq_sb[:, h : h + 1],
                    )
                    yield
                return g()

            def gen_wo(qc, ec_lo, ec_hi, eng=None):
                def g():
                    necs = ec_hi - ec_lo
                    yt = ypool.tile([128, necs, SC], bf16, tag="yt", name="yt")
                    for e4 in range(necs):
                        ec = ec_lo + e4
                        psy = pp.tile([128, SC], f32, tag="pp", name="ps_wo")
                        for h in range(HLOC):
                            nc.tensor.matmul(
                                psy,
                                lhsT=wo_sb[:, h, ec * 128 : (ec + 1) * 128],
                                rhs=ot[:, h, qc * SC : (qc + 1) * SC],
                                start=(h == 0),
                                stop=(h == HLOC - 1),
                            )
                            yield
                        if eng == "act":
                            nc.scalar.copy(yt[:, e4, :], psy)
                        elif eng == "dve":
                            nc.vector.tensor_copy(yt[:, e4, :], psy)
                        else:
                            psum_copy(yt[:, e4, :], psy)
                        yield
                    nc.sync.dma_start(
                        out=yTr[:, ec_lo:ec_hi, qc * SC : (qc + 1) * SC],
                        in_=yt,
                    )
                    yield
                return g()

            from collections import deque

            # pump queue items are (ready_i, generator): steps may only be
            # EMITTED once the post-loop of iteration ready_i-1 has been
            # emitted (cur_i >= ready_i). Emission order defines dependency
            # order in Tile — pulling a Wo filler before the tensor_mul that
            # writes its ot slice is emitted would make it read stale data.
            pump_q = deque()
            cur_i = [0]

            def pump(n):
                while n > 0 and pump_q:
                    ready_i, g = pump_q[0]
                    if ready_i > cur_i[0]:
                        return
                    try:
                        next(g)
                        n -= 1
                    except StopIteration:
                        pump_q.popleft()

            def emit_av_pair(i, j):
                for u in range(2):
                    kj = 2 * j + u
                    nc.tensor.matmul(
                        pso_tiles[i],
                        lhsT=v_sb[:, kj, :],
                        rhs=attn_tiles[i][:, kj, :],
                        start=(kj == 0),
                        stop=(kj == KJT - 1),
                    )

            # pump rate per (qc block, h): tuned so each block's queue
            # drains with a small spill into the next block's h==0
            # iteration, keeping PE fed there with already-safe work.
            RATE = {0: (2, 2, 2, 2), 1: (4, 4, 4, 4), 2: (3, 3, 3, 2),
                    3: (2, 2, 3, 3)}

            # softmax scratch (DVE is strictly in-order, single buffering is
            # safe for everything except rec, which is read one iter later)
            p8 = spool.tile([128, PAIRS, QC], bf16, tag="p8")
            f4 = spool.tile([128, 4, QC], bf16, tag="f4")
            t2 = spool.tile([128, 2, QC], f32, tag="t2")
            acc = spool.tile([128, QC], f32, tag="acc")
            den = spool.tile([128, QC], f32, tag="den")

            for i, (h, qc) in enumerate(iters):
                cur_i[0] = i
                if h == 0:
                    if qc == 0:
                        for hh in range(HLOC):
                            pump_q.append((0, gen_q(hh, 2)))
                    elif qc == 1:
                        for hh in range(HLOC):
                            pump_q.append((0, gen_q(hh, 3)))
                        for ecg in range(8):
                            pump_q.append((5, gen_wo(0, 2 * ecg, 2 * ecg + 2)))
                    elif qc == 2:
                        for ecg in range(8):
                            pump_q.append((9, gen_wo(1, 2 * ecg, 2 * ecg + 2)))
                    else:
                        for ecg in range(6):
                            pump_q.append((13, gen_wo(2, 2 * ecg, 2 * ecg + 2)))

                attn_t = apool.tile([128, KJT, QC], bf16, tag="attn", name="attn")
                attn_tiles[i] = attn_t
                prev = i - 1 if i > 0 else None
                if prev is not None:
                    pso_tiles[prev] = po.tile([128, QC], f32, tag="pso", name="pso")

                q0 = qc * QC
                for j in range(PAIRS):
                    pss = psc.tile([128, 2, QC], f32, tag="pss", name="pss")
                    for u in range(2):
                        kj = 2 * j + u
                        nc.tensor.matmul(
                            pss[:, u, :],
                            lhsT=kt[:, kj * 128 : (kj + 1) * 128],
                            rhs=qt[:, h, q0 : q0 + QC],
                            start=True,
                            stop=True,
                        )
                    if prev is not None:
                        emit_av_pair(prev, j)
                    nc.scalar.activation(
                        attn_t[:, 2 * j : 2 * j + 2, :], pss, AF.Exp,
                        scale=INV_SQRT_HD,
                    )
                    # incremental pair-sum for the softmax denominator
                    nc.vector.tensor_tensor(
                        p8[:, j, :], attn_t[:, 2 * j, :], attn_t[:, 2 * j + 1, :],
                        op=ALU.add,
                    )
                    pump(RATE[qc][h])

                # finish denominator; normalize prev iter now that its AV
                # accumulation (interleaved above) is complete.
                if prev is not None:
                    ph, pqc = iters[prev]
                    nc.vector.tensor_mul(
                        ot[:, ph, pqc * QC : (pqc + 1) * QC],
                        pso_tiles[prev],
                        rec_tiles[prev],
                    )
                nc.vector.tensor_tensor(
                    f4, p8[:, 0:4, :], p8[:, 4:8, :], op=ALU.add
                )
                nc.vector.tensor_tensor(
                    t2, f4[:, 0:2, :], f4[:, 2:4, :], op=ALU.add
                )
                nc.vector.tensor_tensor(
                    acc, t2[:, 0, :], t2[:, 1, :], op=ALU.add
                )
                nc.gpsimd.partition_all_reduce(den, acc, 128, bass_isa.ReduceOp.add)
                rec = spool.tile([128, QC], f32, tag="rec", bufs=2, name="rec")
                nc.vector.reciprocal(rec, den)
                rec_tiles[i] = rec

            # --- drain: AV + normalize + Wo for the last iteration ---
            last = len(iters) - 1
            pso_tiles[last] = po.tile([128, QC], f32, tag="pso", name="pso")
            cur_i[0] = 99
            pump_q.append((0, gen_wo(2, 12, 14)))
            pump_q.append((0, gen_wo(2, 14, 16)))
            for j in range(PAIRS):
                emit_av_pair(last, j)
                pump(3)
            pump(999)
            lh, lqc = iters[last]
            nc.vector.tensor_mul(
                ot[:, lh, lqc * QC : (lqc + 1) * QC],
                pso_tiles[last],
                rec_tiles[last],
            )
            for gen in (
                gen_wo(3, 0, 2),
                gen_wo(3, 2, 4),
                gen_wo(3, 4, 6),
                gen_wo(3, 6, 8),
                gen_wo(3, 8, 10),
                gen_wo(3, 10, 12),
                gen_wo(3, 12, 14),
                gen_wo(3, 14, 15, eng="dve"),
                gen_wo(3, 15, 16, eng="act"),
            ):
                pump_q.append((0, gen))
            pump(999)

    nc.finalize()
    return nc


def _get_nc():
    if "nc" not in _CACHE:
        _CACHE["nc"] = _build()
    return _CACHE["nc"]


def _shard_inputs(x, Wq, bq, Wk, bk, Wv, bv, Wo, bo):
    import ml_dtypes

    bf = ml_dtypes.bfloat16
    x = np.asarray(x, dtype=np.float32)
    Wq = np.asarray(Wq, dtype=np.float32)
    bq = np.asarray(bq, dtype=np.float32)
    Wk = np.asarray(Wk, dtype=np.float32)
    bk = np.asarray(bk, dtype=np.float32)
    Wv = np.asarray(Wv, dtype=np.float32)
    bv = np.asarray(bv, dtype=np.float32)
    Wo = np.asarray(Wo, dtype=np.float32)

    xTb = [
        np.ascontiguousarray(x[b].transpose(1, 0)).astype(bf) for b in range(B)
    ]
    in_maps = []
    for d in range(NCORES):
        b, g = divmod(d, NGROUPS)
        q0, q1 = g * HLOC * HD, (g + 1) * HLOC * HD
        k0, k1 = g * HD, (g + 1) * HD
        in_maps.append(
            {
                "xT": xTb[b],
                "wq": np.ascontiguousarray(Wq[:, q0:q1]).astype(bf),
                "bq": np.ascontiguousarray(bq[q0:q1]),
                "wk": np.ascontiguousarray(Wk[:, k0:k1]).astype(bf),
                "bk": np.ascontiguousarray(bk[k0:k1]),
                "wv": np.ascontiguousarray(Wv[:, k0:k1]).astype(bf),
                "bvr": np.ascontiguousarray(bv[k0:k1]).astype(bf).reshape(1, HD),
                "wo": np.ascontiguousarray(Wo[q0:q1, :]).astype(bf),
            }
        )
    return in_maps


def _unshard(results, bo):
    y = np.empty((B, S, E), dtype=np.float32)
    for b in range(B):
        acc = results[b * NGROUPS]["yT"].astype(np.float32)
        for g in range(1, NGROUPS):
            acc += results[b * NGROUPS + g]["yT"]
        y[b] = acc.transpose(1, 0) + bo[None, :]
    return y


def kernel(x, Wq, bq, Wk, bk, Wv, bv, Wo, bo, **_):
    from concourse.bass_utils import run_bass_kernel_spmd

    nc = _get_nc()
    in_maps = _shard_inputs(x, Wq, bq, Wk, bk, Wv, bv, Wo, bo)
    res = run_bass_kernel_spmd(nc, in_maps, list(range(NCORES)))
    return _unshard(res.results, np.asarray(bo, dtype=np.float32))


# revision 18
# speedup vs baseline: 1.0014x; 1.0014x over previous
"""GroupQueryAttention kernel for 8 Trainium2 NeuronCores.

Problem: B=2, S=2048, E=2048, H=16 heads, G=4 kv-groups, head_dim=128.

Sharding: batch x kv-group. Core d = (batch b = d//4, group g = d%4) owns
the 4 heads of group g for batch b: the 512-column slice of Wq, the
128-column slice of Wk/Wv, and the matching 512-row slice of Wo. This is
the even split of the model's 77.3e9 MACs: 9.67e9 MACs/core (~247us of
PE time at 1 col/cycle). Each core reads x[b]^T only (contraction dim on
partitions) and writes a partial y^T[b] that the host sums over the 4
group-cores of that batch (+bo, transpose).

dtypes: x/weights/q/k/attn/V/o in bf16 (same PE rate as f32r, half the
DMA and SBUF), psum accumulation and y partials f32.

The PE instruction stream is hand-interleaved. Attention iteration i
emits its score matmuls pair-by-pair with the AV matmuls of iteration
i-1, and a fine-grained "pump" drips filler matmuls (deferred Q
projections, Wo output-projection tiles) into every slot so the in-order
PE queue never stalls while the Act engine drains exp()s (Act needs
~1.04us per [128,1024] exp vs 427ns PE fill per score pair). The softmax
denominator is built incrementally on DVE from bf16 pair-sums so the
last iteration's normalization chain is short.
"""

import math

import numpy as np

B = 2
S = 2048
E = 2048
HD = 128
HLOC = 4  # heads per core (one kv group)
NGROUPS = 4
NCORES = 8
ECH = 16  # e-chunks of 128 for contraction
SC = 512  # s-chunk width for projections / Wo moving dim
NSC = S // SC  # 4
QC = 512  # q-chunk width in attention
NQC = S // QC  # 4
KJT = S // 128  # 16 kj tiles
PAIRS = KJT // 2  # 8 score-psum pairs per iteration
INV_SQRT_HD = 1.0 / math.sqrt(HD)

_CACHE = {}


def _build():
    import concourse.bacc as bacc
    import concourse.mybir as mybir
    import concourse.tile as tile

    f32 = mybir.dt.float32
    bf16 = mybir.dt.bfloat16
    AF = mybir.ActivationFunctionType
    ALU = mybir.AluOpType

    nc = bacc.Bacc("TRN2", target_bir_lowering=False, debug=False)

    xT = nc.dram_tensor("xT", [E, S], bf16, kind="ExternalInput").ap()
    wq = nc.dram_tensor("wq", [E, HLOC * HD], bf16, kind="ExternalInput").ap()
    bq = nc.dram_tensor("bq", [HLOC * HD], f32, kind="ExternalInput").ap()
    wk = nc.dram_tensor("wk", [E, HD], bf16, kind="ExternalInput").ap()
    bk = nc.dram_tensor("bk", [HD], f32, kind="ExternalInput").ap()
    wv = nc.dram_tensor("wv", [E, HD], bf16, kind="ExternalInput").ap()
    bvr = nc.dram_tensor("bvr", [1, HD], bf16, kind="ExternalInput").ap()
    wo = nc.dram_tensor("wo", [HLOC * HD, E], bf16, kind="ExternalInput").ap()
    yT = nc.dram_tensor("yT", [E, S], bf16, kind="ExternalOutput").ap()

    import bass_rust  # noqa: F401
    from concourse import bass_isa

    xTr = xT.rearrange("(t p) s -> p t s", p=128)
    yTr = yT.rearrange("(t p) s -> p t s", p=128)

    with tile.TileContext(nc) as tc:
        with (
            tc.tile_pool(name="pers", bufs=1) as pers,
            tc.tile_pool(name="xt", bufs=3) as xpool,
            tc.tile_pool(name="proj", bufs=1) as projp,
            tc.tile_pool(name="attn", bufs=2) as apool,
            tc.tile_pool(name="soft", bufs=1) as spool,
            tc.tile_pool(name="yst", bufs=3) as ypool,
            tc.tile_pool(name="ps_pp", bufs=2, space="PSUM") as pp,
            tc.tile_pool(name="ps_sc", bufs=2, space="PSUM") as psc,
            tc.tile_pool(name="ps_o", bufs=2, space="PSUM") as po,
        ):
            # --- persistent tiles ---
            wk_sb = pers.tile([128, ECH, HD], bf16)
            wv_sb = pers.tile([128, ECH, HD], bf16)
            bk_sb = pers.tile([128, 1], f32)
            bvr_sb = pers.tile([1, HD], bf16)
            wq_sb = pers.tile([128, ECH, HLOC * HD], bf16)
            bq_sb = pers.tile([128, HLOC], f32)
            wo_sb = pers.tile([128, HLOC, E], bf16)
            ones_sb = pers.tile([1, 128], bf16)
            nc.vector.memset(ones_sb, 1.0)

            # --- per-core activations ---
            qt = projp.tile([128, HLOC, S], bf16, tag="qt")
            kt = projp.tile([128, S], bf16, tag="kt")
            v_sb = projp.tile([128, KJT, HD], bf16, tag="v")
            ot = projp.tile([128, HLOC, S], bf16, tag="ot")

            copy_flip = [0]

            def psum_copy(dst, src):
                if copy_flip[0] % 2 == 0:
                    nc.scalar.copy(dst, src)
                else:
                    nc.vector.tensor_copy(dst, src)
                copy_flip[0] += 1

            xts = {}

            def load_x(sc, halves=1):
                t = xpool.tile([128, ECH, SC], bf16, tag="xt", name="xt")
                hh = ECH // halves
                for u in range(halves):
                    nc.sync.dma_start(
                        out=t[:, u * hh : (u + 1) * hh, :],
                        in_=xTr[:, u * hh : (u + 1) * hh, sc * SC : (sc + 1) * SC],
                    )
                xts[sc] = t

            def q_proj(h, qcn):
                ps = pp.tile([128, SC], f32, tag="pp", name="ps_q")
                xt_t = xts[qcn]
                for t in range(ECH):
                    nc.tensor.matmul(
                        ps,
                        lhsT=wq_sb[:, t, h * HD : (h + 1) * HD],
                        rhs=xt_t[:, t, :],
                        start=(t == 0),
                        stop=(t == ECH - 1),
                    )
                nc.scalar.activation(
                    qt[:, h, qcn * SC : (qcn + 1) * SC], ps, AF.Identity,
                    bias=bq_sb[:, h : h + 1],
                )

            def kv_unit(sc):
                xt_t = xts[sc]
                s0 = sc * SC
                ps = pp.tile([128, SC], f32, tag="pp", name="ps_k")
                for t in range(ECH):
                    nc.tensor.matmul(
                        ps,
                        lhsT=wk_sb[:, t, :],
                        rhs=xt_t[:, t, :],
                        start=(t == 0),
                        stop=(t == ECH - 1),
                    )
                nc.scalar.activation(
                    kt[:, s0 : s0 + SC], ps, AF.Identity, bias=bk_sb[:, 0:1]
                )
                # V directly in [s, hd] layout: x-tile is lhsT, wv is rhs;
                # bv folded in via a ones-row matmul (bias varies along the
                # free axis here, so the Act bias port can't add it).
                for j in range(SC // 128):
                    st = sc * (SC // 128) + j
                    psv = pp.tile([128, SC], f32, tag="pp", name="ps_v")
                    for t in range(ECH):
                        nc.tensor.matmul(
                            psv[:, 0:HD],
                            lhsT=xt_t[:, t, j * 128 : (j + 1) * 128],
                            rhs=wv_sb[:, t, :],
                            start=(t == 0),
                            stop=False,
                        )
                    nc.tensor.matmul(
                        psv[:, 0:HD], lhsT=ones_sb, rhs=bvr_sb,
                        start=False, stop=True,
                    )
                    nc.scalar.copy(v_sb[:, st, :], psv[:, 0:HD])

            # --- phase 1: K/V for all of S, Q for chunks 0-1 ---
            # DMA issue order is tuned so each consumer's data lands just
            # ahead of its matmuls (single SP HWDGE queue = bus order).
            xt0 = xpool.tile([128, ECH, SC], bf16, tag="xt", name="xt0")
            xts[0] = xt0
            nc.sync.dma_start(out=xt0[:, 0:4, :], in_=xTr[:, 0:4, 0:SC])
            nc.sync.dma_start(out=wk_sb, in_=wk.rearrange("(t p) m -> p t m", p=128))
            nc.sync.dma_start(out=xt0[:, 4:8, :], in_=xTr[:, 4:8, 0:SC])
            nc.sync.dma_start(out=xt0[:, 8:16, :], in_=xTr[:, 8:16, 0:SC])
            nc.sync.dma_start(out=wv_sb, in_=wv.rearrange("(t p) m -> p t m", p=128))
            nc.sync.dma_start(out=bk_sb, in_=bk.rearrange("(d o) -> d o", o=1))
            nc.sync.dma_start(out=bvr_sb, in_=bvr)
            load_x(1, halves=2)
            wqr = wq.rearrange("(t p) m -> p t m", p=128)
            nc.sync.dma_start(out=wq_sb[:, :, 0 : 2 * HD], in_=wqr[:, :, 0 : 2 * HD])
            nc.sync.dma_start(
                out=wq_sb[:, :, 2 * HD : 4 * HD], in_=wqr[:, :, 2 * HD : 4 * HD]
            )
            nc.sync.dma_start(out=bq_sb, in_=bq.rearrange("(h d) -> d h", d=128))

            kv_unit(0)
            load_x(2, halves=2)
            nc.sync.dma_start(out=wo_sb, in_=wo.rearrange("(h p) e -> p h e", p=128))
            kv_unit(1)
            for h in range(HLOC):
                q_proj(h, 0)
            kv_unit(2)
            load_x(3, halves=2)
            for h in range(HLOC):
                q_proj(h, 1)
            kv_unit(3)

            # --- phase 2: attention pipeline with pumped fillers ---
            iters = [(h, qc) for qc in range(NQC) for h in range(HLOC)]
            attn_tiles = {}
            pso_tiles = {}
            rec_tiles = {}

            def gen_q(h, qcn):
                def g():
                    ps = pp.tile([128, SC], f32, tag="pp", name="ps_qf")
                    xt_t = xts[qcn]
                    for t in range(ECH):
                        nc.tensor.matmul(
                            ps,
                            lhsT=wq_sb[:, t, h * HD : (h + 1) * HD],
                            rhs=xt_t[:, t, :],
                            start=(t == 0),
                            stop=(t == ECH - 1),
                        )
                        yield
                    nc.scalar.activation(
                        qt[:, h, qcn * SC : (qcn + 1) * SC], ps, AF.Identity,
                        bias=bq_sb[:, h : h + 1],
                    )
                    yield
                return g()

            def gen_wo(qc, ec_lo, ec_hi, eng=None):
                def g():
                    necs = ec_hi - ec_lo
                    yt = ypool.tile([128, necs, SC], bf16, tag="yt", name="yt")
                    for e4 in range(necs):
                        ec = ec_lo + e4
                        psy = pp.tile([128, SC], f32, tag="pp", name="ps_wo")
                        for h in range(HLOC):
                            nc.tensor.matmul(
                                psy,
                                lhsT=wo_sb[:, h, ec * 128 : (ec + 1) * 128],
                                rhs=ot[:, h, qc * SC : (qc + 1) * SC],
                                start=(h == 0),
                                stop=(h == HLOC - 1),
                            )
                            yield
                        if eng == "act":
                            nc.scalar.copy(yt[:, e4, :], psy)
                        elif eng == "dve":
                            nc.vector.tensor_copy(yt[:, e4, :], psy)
                        else:
                            psum_copy(yt[:, e4, :], psy)
                        yield
                    nc.sync.dma_start(
                        out=yTr[:, ec_lo:ec_hi, qc * SC : (qc + 1) * SC],
                        in_=yt,
                    )
                    yield
                return g()

            from collections import deque

            # pump queue items are (ready_i, generator): steps may only be
            # EMITTED once the post-loop of iteration ready_i-1 has been
            # emitted (cur_i >= ready_i). Emission order defines dependency
            # order in Tile — pulling a Wo filler before the tensor_mul that
            # writes its ot slice is emitted would make it read stale data.
            pump_q = deque()
            cur_i = [0]

            def pump(n):
                while n > 0 and pump_q:
                    ready_i, g = pump_q[0]
                    if ready_i > cur_i[0]:
                        return
                    try:
                        next(g)
                        n -= 1
                    except StopIteration:
                        pump_q.popleft()

            def emit_av_pair(i, j):
                for u in range(2):
                    kj = 2 * j + u
                    nc.tensor.matmul(
                        pso_tiles[i],
                        lhsT=v_sb[:, kj, :],
                        rhs=attn_tiles[i][:, kj, :],
                        start=(kj == 0),
                        stop=(kj == KJT - 1),
                    )

            # pump rate per (qc block, h): tuned so each block's queue
            # drains with a small spill into the next block's h==0
            # iteration, keeping PE fed there with already-safe work.
            RATE = {0: (2, 2, 2, 2), 1: (4, 4, 4, 4), 2: (3, 3, 3, 2),
                    3: (2, 2, 3, 3)}

            # softmax scratch (DVE is strictly in-order, single buffering is
            # safe for everything except rec, which is read one iter later)
            p8 = spool.tile([128, PAIRS, QC], bf16, tag="p8")
            f4 = spool.tile([128, 4, QC], bf16, tag="f4")
            t2 = spool.tile([128, 2, QC], f32, tag="t2")
            acc = spool.tile([128, QC], f32, tag="acc")
            den = spool.tile([128, QC], f32, tag="den")

            for i, (h, qc) in enumerate(iters):
                cur_i[0] = i
                if h == 0:
                    if qc == 0:
                        for hh in range(HLOC):
                            pump_q.append((0, gen_q(hh, 2)))
                    elif qc == 1:
                        for hh in range(HLOC):
                            pump_q.append((0, gen_q(hh, 3)))
                        for ecg in range(8):
                            pump_q.append((5, gen_wo(0, 2 * ecg, 2 * ecg + 2)))
                    elif qc == 2:
                        for ecg in range(8):
                            pump_q.append((9, gen_wo(1, 2 * ecg, 2 * ecg + 2)))
                    else:
                        for ecg in range(6):
                            pump_q.append((13, gen_wo(2, 2 * ecg, 2 * ecg + 2)))

                attn_t = apool.tile([128, KJT, QC], bf16, tag="attn", name="attn")
                attn_tiles[i] = attn_t
                prev = i - 1 if i > 0 else None
                if prev is not None:
                    pso_tiles[prev] = po.tile([128, QC], f32, tag="pso", name="pso")

                q0 = qc * QC
                for j in range(PAIRS):
                    pss = psc.tile([128, 2, QC], f32, tag="pss", name="pss")
                    for u in range(2):
                        kj = 2 * j + u
                        nc.tensor.matmul(
                            pss[:, u, :],
                            lhsT=kt[:, kj * 128 : (kj + 1) * 128],
                            rhs=qt[:, h, q0 : q0 + QC],
                            start=True,
                            stop=True,
                        )
                    if prev is not None:
                        emit_av_pair(prev, j)
                    nc.scalar.activation(
                        attn_t[:, 2 * j : 2 * j + 2, :], pss, AF.Exp,
                        scale=INV_SQRT_HD,
                    )
                    # incremental pair-sum for the softmax denominator
                    nc.vector.tensor_tensor(
                        p8[:, j, :], attn_t[:, 2 * j, :], attn_t[:, 2 * j + 1, :],
                        op=ALU.add,
                    )
                    pump(RATE[qc][h])

                # finish denominator; normalize prev iter now that its AV
                # accumulation (interleaved above) is complete.
                if prev is not None:
                    ph, pqc = iters[prev]
                    nc.vector.tensor_mul(
                        ot[:, ph, pqc * QC : (pqc + 1) * QC],
                        pso_tiles[prev],
                        rec_tiles[prev],
                    )
                nc.vector.tensor_tensor(
                    f4, p8[:, 0:4, :], p8[:, 4:8, :], op=ALU.add
                )
                nc.vector.tensor_tensor(
                    t2, f4[:, 0:2, :], f4[:, 2:4, :], op=ALU.add
                )
                nc.vector.tensor_tensor(
                    acc, t2[:, 0, :], t2[:, 1, :], op=ALU.add
                )
                nc.gpsimd.partition_all_reduce(den, acc, 128, bass_isa.ReduceOp.add)
                rec = spool.tile([128, QC], f32, tag="rec", bufs=2, name="rec")
                nc.vector.reciprocal(rec, den)
                rec_tiles[i] = rec

            # --- drain: AV + normalize + Wo for the last iteration ---
            last = len(iters) - 1
            pso_tiles[last] = po.tile([128, QC], f32, tag="pso", name="pso")
            cur_i[0] = 99
            pump_q.append((0, gen_wo(2, 12, 14)))
            pump_q.append((0, gen_wo(2, 14, 16)))
            for j in range(PAIRS):
                emit_av_pair(last, j)
                pump(3)
            pump(999)
            lh, lqc = iters[last]
            nc.vector.tensor_mul(
                ot[:, lh, lqc * QC : (lqc + 1) * QC],
                pso_tiles[last],
                rec_tiles[last],
            )
            for gen in (
                gen_wo(3, 0, 2),
                gen_wo(3, 2, 4),
                gen_wo(3, 4, 6),
                gen_wo(3, 6, 8),
                gen_wo(3, 8, 10),
                gen_wo(3, 10, 12),
                gen_wo(3, 12, 14),
                gen_wo(3, 14, 15, eng="dve"),
                gen_wo(3, 15, 16, eng="act"),
            ):
                pump_q.append((0, gen))
            pump(999)

    nc.finalize()
    return nc


def _get_nc():
    if "nc" not in _CACHE:
        _CACHE["nc"] = _build()
    return _CACHE["nc"]


def _shard_inputs(x, Wq, bq, Wk, bk, Wv, bv, Wo, bo):
    import ml_dtypes

    bf = ml_dtypes.bfloat16
    x = np.asarray(x, dtype=np.float32)
    Wq = np.asarray(Wq, dtype=np.float32)
    bq = np.asarray(bq, dtype=np.float32)
    Wk = np.asarray(Wk, dtype=np.float32)
    bk = np.asarray(bk, dtype=np.float32)
    Wv = np.asarray(Wv, dtype=np.float32)
    bv = np.asarray(bv, dtype=np.float32)
    Wo = np.asarray(Wo, dtype=np.float32)

    xTb = [
        np.ascontiguousarray(x[b].transpose(1, 0)).astype(bf) for b in range(B)
    ]
    in_maps = []
    for d in range(NCORES):
        b, g = divmod(d, NGROUPS)
        q0, q1 = g * HLOC * HD, (g + 1) * HLOC * HD
        k0, k1 = g * HD, (g + 1) * HD
        in_maps.append(
            {
                "xT": xTb[b],
                "wq": np.ascontiguousarray(Wq[:, q0:q1]).astype(bf),
                "bq": np.ascontiguousarray(bq[q0:q1]),
                "wk": np.ascontiguousarray(Wk[:, k0:k1]).astype(bf),
                "bk": np.ascontiguousarray(bk[k0:k1]),
                "wv": np.ascontiguousarray(Wv[:, k0:k1]).astype(bf),
                "bvr": np.ascontiguousarray(bv[k0:k1]).astype(bf).reshape(1, HD),
                "wo": np.ascontiguousarray(Wo[q0:q1, :]).astype(bf),
            }
        )
    return in_maps


def _unshard(results, bo):
    y = np.empty((B, S, E), dtype=np.float32)
    for b in range(B):
        acc = results[b * NGROUPS]["yT"].astype(np.float32)
        for g in range(1, NGROUPS):
            acc += results[b * NGROUPS + g]["yT"]
        y[b] = acc.transpose(1, 0) + bo[None, :]
    return y


def kernel(x, Wq, bq, Wk, bk, Wv, bv, Wo, bo, **_):
    from concourse.bass_utils import run_bass_kernel_spmd

    nc = _get_nc()
    in_maps = _shard_inputs(x, Wq, bq, Wk, bk, Wv, bv, Wo, bo)
    res = run_bass_kernel_spmd(nc, in_maps, list(range(NCORES)))
    return _unshard(res.results, np.asarray(bo, dtype=np.float32))
